# revision 42
# baseline (speedup 1.0000x reference)
"""GQA causal attention (RoPE, B=2 S=2048 D=2048 H=16 KV=8 HD=128) on 8 trn2 cores.

Strategy: head-parallel. Each core c owns q-heads {2c, 2c+1} and kv-head c.
Host replicates x (packed per-chunk contiguous, bf16) to all cores; all
projections, RoPE and causal attention are head-sharded (zero comm). Two
AllToAlls (one per local q-head, 1 MB/rank each, bf16) convert the attention
output from head-sharding to row-sharding overlapped with the other head's
attention, then each core computes its 512-row slice of the output projection
with the full Wo. Host concatenates the 8 row shards.

Perf notes (450us baseline -> ~361us):
- All inputs are host-packed so each DMA has multi-KB contiguous lines
  (descriptor-rate, ~43ns/1KB-line/engine, was the phase-1 bottleneck).
- Attention is software-pipelined: the PV matmuls of iteration i-1 are woven
  tile-by-tile between the score matmuls of iteration i so the scalar
  engine's exp keeps up and PSUM score banks recycle in time. qc runs
  ascending on even b / descending on odd b so adjacent iterations have
  matched tile counts (no bare score runs at block boundaries).
- The softmax denominator is a bf16 tree-sum of prob tiles on the (otherwise
  idle) vector engine plus one short ones-matmul per chunk; the causal mask
  is applied post-exp as a 0/1 multiply on DVE (raw masked-region scores are
  bounded, exp cannot overflow) - both remove ~90k cycles of PE work.
- Output projection accumulates the 8 heads from AllToAll#0 for all four row
  tiles first (hiding AllToAll#1 behind ~27us of PE work), drains partials
  to SBUF, then adds the second-half chains with a DVE combine.
- The PE is power-throttled (GPIO brake) to ~1.2-2GHz under sustained load,
  and cross-core launch skew (20-50us) surfaces as AllToAll wait time;
  both cap further gains from scheduling alone.
"""

import os
import sys

import numpy as np

if "/opt/trn_rl_repo" not in sys.path:
    sys.path.insert(0, "/opt/trn_rl_repo")

CORES = 8


def build_nc(B, S, D, H, KV, HD, HO, QC):
    """Build the SPMD bass graph (same graph for all 8 cores)."""
    import concourse.bacc as bacc
    import concourse.tile as tile
    from concourse import mybir
    from contextlib import ExitStack

    f32 = mybir.dt.float32
    bf16 = mybir.dt.bfloat16
    ACT = mybir.ActivationFunctionType

    QH = H // CORES               # q heads per core (2)
    R = B * S                     # total rows (4096)
    RO = R // CORES               # output rows per core (512) == QC
    assert QC == RO
    DK = D // 128                 # k-tiles over model dim (16)
    RC = 512                      # row-chunk width for projections
    NCH = R // RC                 # projection row chunks (8)
    NQC = S // QC                 # q chunks per batch (4)
    NKT = S // 128                # k tiles per batch (16)
    NT = QC // 128                # diagonal mask patterns (4)
    NRT = RO // 128               # out row tiles per core (4)
    OC = min(D, 512)              # out col chunk
    NOC = D // OC                 # out col chunks (4)
    HG = H                        # total heads in O-proj
    WC = QH * HD + 2 * HD         # packed projection cols (512)
    scale = float(HD) ** -0.5

    nc = bacc.Bacc("TRN2", target_bir_lowering=False, debug=False,
                   num_devices=CORES)

    xP = nc.dram_tensor("xP", [128, NCH, DK, RC], bf16, kind="ExternalInput")
    cosT = nc.dram_tensor("cosT", [HD, S], bf16, kind="ExternalInput")
    sinT = nc.dram_tensor("sinT", [HD, S], bf16, kind="ExternalInput")
    wPack = nc.dram_tensor("wPack", [128, DK, WC], bf16, kind="ExternalInput")
    wo = nc.dram_tensor("wo", [HO, D], bf16, kind="ExternalInput")
    masks = nc.dram_tensor("masks", [128, NT * QC], bf16, kind="ExternalInput")
    ident = nc.dram_tensor("ident", [128, 128], bf16, kind="ExternalInput")
    ones = nc.dram_tensor("ones", [128, 128], bf16, kind="ExternalInput")
    out = nc.dram_tensor("out", [RO, D], bf16, kind="ExternalOutput")

    with tile.TileContext(nc) as tc, ExitStack() as top:
        dram = top.enter_context(tc.tile_pool(name="dram", bufs=1, space="DRAM"))
        consts = top.enter_context(tc.tile_pool(name="consts", bufs=1))
        resid = top.enter_context(tc.tile_pool(name="resid", bufs=1))

        a2a_in = [dram.tile([CORES, 128, QC], bf16, name=f"a2ain{h}")
                  for h in range(QH)]
        a2a_out = [dram.tile([CORES, 128, QC], bf16, name=f"a2aout{h}")
                   for h in range(QH)]

        ident_sb = consts.tile([128, 128], bf16)
        ones_sb = consts.tile([128, 128], bf16)
        mask_sb = consts.tile([128, NT * QC], bf16)

        # residents produced by projection phase, consumed by attention
        qT_sb = resid.tile([128, QH, R], bf16)       # [hd, head, row]
        kT_sb = resid.tile([128, R], bf16)           # [hd, row]
        v_sb = resid.tile([128, R], bf16)            # [kpos%128, ktile*HD+hd]

        # full Wo resident (DMA'd during phase 2; no contention with x)
        wo_all = resid.tile([128, HG, D], bf16)
        attn_all = resid.tile([128, HG, QC], bf16)

        # ------------------------------- phase 1: projections + rope
        with ExitStack() as ph1:
            ropec = ph1.enter_context(tc.tile_pool(name="ropec", bufs=1))
            wpool = ph1.enter_context(tc.tile_pool(name="wpool", bufs=1))
            xpool = ph1.enter_context(tc.tile_pool(name="xpool", bufs=3))
            rtmp = ph1.enter_context(tc.tile_pool(name="rtmp", bufs=2))
            vtp = ph1.enter_context(tc.tile_pool(name="vtp", bufs=2))
            psA = ph1.enter_context(tc.tile_pool(name="psA", bufs=4, space="PSUM"))
            psTR = ph1.enter_context(tc.tile_pool(name="psTR", bufs=2, space="PSUM"))

            w_sb = wpool.tile([128, DK, WC], bf16)
            cos_sb = ropec.tile([128, S], bf16)
            sin_sb = ropec.tile([128, S], bf16)

            # chunk 0 + weights interleaved in quarter-pieces so the first
            # matmul can start after ~2 pieces land (subtile deps); few
            # trigger instructions keep the sync queue ahead of the PE
            G4 = max(DK // 4, 1)
            xch0 = xpool.tile([128, DK, RC], bf16, tag="xch", name="xch0")
            for gi, g in enumerate(range(0, DK, G4)):
                sl = slice(g, g + G4)
                nc.sync.dma_start(out=w_sb[:, sl, :], in_=wPack[:, sl, :])
                nc.sync.dma_start(out=xch0[:, sl, :], in_=xP[:, 0, sl, :])
                if gi == 1:
                    nc.sync.dma_start(out=cos_sb, in_=cosT[:, :])
                    nc.sync.dma_start(out=sin_sb, in_=sinT[:, :])
            nc.sync.dma_start(out=ident_sb, in_=ident[:, :])
            # remaining chunks: one DMA each (16KB contiguous lines);
            # Wo trickles in behind them (keeps the a2a window DMA-quiet)
            wo_r = wo.ap().rearrange("(g p) n -> p g n", p=128)
            wo_done = set()
            xchs = [xch0]
            for n in range(1, NCH):
                xc = xpool.tile([128, DK, RC], bf16, tag="xch", name="xch")
                nc.sync.dma_start(out=xc, in_=xP[:, n, :, :])
                xchs.append(xc)
                if n == min(2, NCH - 1):
                    nc.sync.dma_start(out=ones_sb, in_=ones[:, :])
                    nc.sync.dma_start(out=mask_sb, in_=masks[:, :])
                # last two Wo quarters ride behind the final x chunks (x DMA
                # traffic has drained by then); the rest go at phase-2 start
                q4 = n - (NCH - 2)
                if 0 <= q4 < 2:
                    wo_done.add(q4)
                    sl = slice(q4 * HG // 4, (q4 + 1) * HG // 4)
                    nc.sync.dma_start(out=wo_all[:, sl, :], in_=wo_r[:, sl, :])
            wo_rest = [q4 for q4 in range(4) if q4 not in wo_done]

            half = HD // 2

            def rope(pp, dst, poff):
                c_sl = cos_sb[:, poff:poff + RC]
                s_sl = sin_sb[:, poff:poff + RC]
                t1 = rtmp.tile([128, RC], f32, tag="t1", name="t1")
                t2 = rtmp.tile([128, RC], f32, tag="t2", name="t2")
                nc.vector.tensor_mul(t1, pp, c_sl)
                nc.vector.tensor_mul(t2[0:half, :], pp[half:128, :], s_sl[0:half, :])
                nc.vector.tensor_mul(t2[half:128, :], pp[0:half, :], s_sl[half:128, :])
                nc.vector.tensor_add(dst, t1, t2)

            pend_tr = []   # deferred PE transposes from the previous chunk
            for n in range(NCH):
                xs = xchs[n]
                poff = (n * RC) % S
                ois = list(range(QH + 2))  # QH q heads, then k, then vT
                if n == NCH - 1:
                    # last chunk: v first so its transposes (emitted at loop
                    # end) don't stall the PE at the phase boundary
                    ois = [QH + 1] + ois[:-1]
                for ji, oi in enumerate(ois):
                    pp = psA.tile([128, RC], f32, tag="pp", name="pp")
                    coff = oi * HD if oi < QH else (QH + (oi - QH)) * HD
                    for kt in range(DK):
                        nc.tensor.matmul(
                            pp, lhsT=w_sb[:, kt, coff:coff + HD],
                            rhs=xs[:, kt, :],
                            start=(kt == 0), stop=(kt == DK - 1))
                    if ji == 0 and pend_tr:
                        for t in pend_tr:
                            t()
                        pend_tr = []
                    if oi < QH:
                        rope(pp, qT_sb[:, oi, n * RC:(n + 1) * RC], poff)
                    elif oi == QH:
                        rope(pp, kT_sb[:, n * RC:(n + 1) * RC], poff)
                    else:
                        vt_sb = vtp.tile([128, RC], bf16, tag="vt", name="vt")
                        nc.scalar.activation(vt_sb, pp, ACT.Copy)

                        def mk(nn, vt):
                            def emit():
                                for j in range(RC // 128):
                                    ptr_ = psTR.tile([128, 128], bf16,
                                                     tag="ptr", name="ptr")
                                    nc.tensor.transpose(
                                        ptr_, vt[:, j * 128:(j + 1) * 128],
                                        ident_sb)
                                    rti = nn * (RC // 128) + j
                                    nc.scalar.activation(
                                        v_sb[:, rti * 128:(rti + 1) * 128],
                                        ptr_, ACT.Copy)
                            return emit
                        pend_tr.append(mk(n, vt_sb))
            for t in pend_tr:
                t()
            pend_tr = []

        # ------------------------------- phase 2: attention, pipelined
        with ExitStack() as ph2:
            probs = ph2.enter_context(tc.tile_pool(name="probs", bufs=46))
            atmp = ph2.enter_context(tc.tile_pool(name="atmp", bufs=3))
            dens = ph2.enter_context(tc.tile_pool(name="dens", bufs=2))
            accp = ph2.enter_context(tc.tile_pool(name="accp", bufs=4))
            psS = ph2.enter_context(tc.tile_pool(name="psS", bufs=3, space="PSUM"))
            psO = ph2.enter_context(tc.tile_pool(name="psO", bufs=3, space="PSUM"))
            psD = ph2.enter_context(tc.tile_pool(name="psD", bufs=1, space="PSUM"))
            psB = ph2.enter_context(tc.tile_pool(name="psB", bufs=1, space="PSUM"))

            from concourse import mybir as _mb

            for q4 in wo_rest:   # remaining Wo quarters (x traffic is done)
                sl = slice(q4 * HG // 4, (q4 + 1) * HG // 4)
                nc.sync.dma_start(out=wo_all[:, sl, :], in_=wo_r[:, sl, :])

            # qc ascending on even b, descending on odd b: adjacent
            # iterations then have matched tile counts (the PV weave of the
            # previous iteration fills every score position, so exp on the
            # scalar engine never falls behind), while each h still ends on
            # a small qc for a fast AllToAll trigger.
            iters = []
            for h in range(QH):
                for b in range(B):
                    qcs = (range(NQC) if b % 2 == 0
                           else range(NQC - 1, -1, -1))
                    iters.extend((h, b, qc) for qc in qcs)

            def score_tile(it, kt):
                """Emit the score matmul, exp, post-exp mask (DVE), and the
                DVE acc-chain add for tile kt of it.

                Unmasked scores are bounded (|s*scale| < ~6 for these
                magnitudes) so exp can run on the raw scores; the causal mask
                is applied post-exp as a 0/1 multiply on the vector engine,
                keeping the PE free of mask matmuls."""
                h, b, qc = it["h"], it["b"], it["qc"]
                dj = kt - qc * NT
                o = max(dj, 0) * 128
                kl = kT_sb[:, b * S + kt * 128: b * S + (kt + 1) * 128]
                sc = psS.tile([128, QC], f32, tag="sc", name="sc")
                nc.tensor.matmul(
                    sc[:, o:QC], lhsT=kl,
                    rhs=qT_sb[:, h, b * S + qc * QC + o: b * S + (qc + 1) * QC],
                    start=True, stop=True)
                pr = probs.tile([128, QC], bf16, tag="pr", name="pr")
                nc.scalar.activation(pr[:, o:QC], sc[:, o:QC], ACT.Exp,
                                     scale=scale)
                if dj >= 0:
                    prm = probs.tile([128, QC], bf16, tag="pr", name="prm")
                    nc.vector.tensor_mul(prm[:, o:QC], pr[:, o:QC],
                                         mask_sb[:, dj * QC + o:(dj + 1) * QC])
                    pr = prm
                it["prs"][kt] = pr
                it["offs"][kt] = o
                # denominator partial sums on DVE (bf16, 2x rate):
                # acc[kp, q] = sum_kt pr_kt[kp, q]; final row-sum is one
                # short PE matmul per chunk.
                if kt == 0:
                    it["acc"] = pr          # region [0:QC] valid (o == 0)
                else:
                    po_ = it["offs"][kt]
                    na = accp.tile([128, QC], bf16, tag="acc", name="acc")
                    if po_ > 0:
                        nc.vector.tensor_copy(na[:, 0:po_], it["acc"][:, 0:po_])
                    nc.vector.tensor_add(na[:, po_:QC], it["acc"][:, po_:QC],
                                         pr[:, po_:QC])
                    it["acc"] = na

            def pv_tile(it, kt):
                """Emit the PV matmul for tile kt of it."""
                b, nkt = it["b"], it["nkt"]
                o = it["offs"][kt]
                ktg = b * NKT + kt
                nc.tensor.matmul(
                    it["po"][:, o:QC], lhsT=v_sb[:, ktg * 128:(ktg + 1) * 128],
                    rhs=it["prs"][kt][:, o:QC],
                    start=(kt == 0), stop=(kt == nkt - 1))

            def den_stage(it):
                """One short den matmul on the acc, then a bf16 round of the
                denominator. Everything off the scalar engine: ACT must only
                run exp, or the copies delay the exp stream and stall the PV
                weave. The reciprocal happens AFTER the broadcast (on the
                [128, QC] tile) so the whole chain is one DVE op shorter."""
                pden = psD.tile([1, QC], f32, tag="pden", name="pden")
                nc.tensor.matmul(pden, lhsT=ones_sb[:, 0:1], rhs=it["acc"],
                                 start=True, stop=True)
                den_b = dens.tile([1, QC], bf16, tag="denb", name="den_b")
                nc.vector.tensor_copy(den_b, pden)
                it["den_b"] = den_b

            def norm_stage(it):
                """broadcast den (PE) -> reciprocal on the broadcast (DVE)."""
                pbc = psB.tile([128, QC], f32, tag="pbc", name="pbc")
                nc.tensor.matmul(pbc, lhsT=ones_sb[0:1, :], rhs=it["den_b"],
                                 start=True, stop=True)
                rec = atmp.tile([128, QC], f32, tag="rec", name="rec")
                nc.vector.reciprocal_approx_fast(rec, pbc)
                it["rec"] = rec

            def norm_stage2(it):
                """scale po by 1/den (DVE) -> a2a-in DMA."""
                h, b, qc = it["h"], it["b"], it["qc"]
                anorm = atmp.tile([128, QC], bf16, tag="an", name="anorm")
                nc.vector.tensor_mul(anorm, it["po"], it["rec"])
                d = b * NQC + qc
                nc.sync.dma_start(out=a2a_in[h][d], in_=anorm)
                if it["h_last"]:
                    finish_h(h)

            def finish_h(h):
                nc.gpsimd.collective_compute(
                    "AllToAll", _mb.AluOpType.bypass,
                    ins=[a2a_in[h].opt()], outs=[a2a_out[h].opt()],
                    replica_groups=[list(range(CORES))])
                asrc = a2a_out[h].rearrange("g p q -> p g q")
                adst = attn_all.rearrange("p (g hl) q -> p g hl q", hl=QH)
                for q8 in range(min(8, CORES)):
                    gs = max(CORES // 8, 1)
                    sl = slice(q8 * gs, (q8 + 1) * gs)
                    if sl.start >= CORES:
                        break
                    nc.sync.dma_start(out=adst[:, sl, h, :], in_=asrc[:, sl, :])

            prev = None   # iteration whose PV/den/norm weave into the current
            for ii, (h, b, qc) in enumerate(iters):
                cur = {"h": h, "b": b, "qc": qc, "nkt": (qc + 1) * NT,
                       "prs": {}, "offs": {},
                       "h_last": (ii == len(iters) - 1
                                  or iters[ii + 1][0] != h),
                       "po": psO.tile([128, QC], f32, tag="po", name="po")}
                nw = max(cur["nkt"], prev["nkt"] if prev else 0)
                den_done = norm1_done = norm2_done = prev is None
                for i in range(nw):
                    if i < cur["nkt"]:
                        score_tile(cur, i)
                    if prev and i < prev["nkt"]:
                        pv_tile(prev, i)
                    if i == 2 and not den_done:
                        den_stage(prev)
                        den_done = True
                    if den_done and not norm1_done and i >= 6:
                        norm_stage(prev)
                        norm1_done = True
                    # the final scale may only run once prev's PV chain has
                    # fully accumulated (pv_tile(prev, nkt-1) emitted above)
                    if (norm1_done and not norm2_done and prev
                            and i >= max(prev["nkt"] - 1, 8)):
                        norm_stage2(prev)
                        norm2_done = True
                if not den_done:
                    den_stage(prev)
                if not norm1_done:
                    norm_stage(prev)
                if not norm2_done:
                    norm_stage2(prev)
                prev = cur
            # flush: PV of the last iteration, then its den/norm
            for i in range(prev["nkt"]):
                pv_tile(prev, i)
            den_stage(prev)
            norm_stage(prev)
            norm_stage2(prev)

        # ------------------------------- phase 3: output projection
        # Pass A: accumulate the QH*? heads delivered by AllToAll#0 for all
        # row tiles (this work hides AllToAll#1), drain partials to SBUF.
        # Pass B: second-half chains + DVE combine -> bf16 out.
        with ExitStack() as ph3:
            parts = ph3.enter_context(tc.tile_pool(name="parts", bufs=NRT))
            outp = ph3.enter_context(tc.tile_pool(name="outp", bufs=2))
            psP = ph3.enter_context(tc.tile_pool(name="psP", bufs=2, space="PSUM"))

            gsA = [g for g in range(HG) if g % QH == 0]
            gsB = [g for g in range(HG) if g % QH != 0]
            partA = []
            for rt in range(NRT):
                pp = [psP.tile([128, OC], f32, tag=f"ppo{oc}", name=f"ppo{oc}")
                      for oc in range(NOC)]
                for gi, g in enumerate(gsA):
                    al = attn_all[:, g, rt * 128:(rt + 1) * 128]
                    for oc in range(NOC):
                        nc.tensor.matmul(
                            pp[oc], lhsT=al,
                            rhs=wo_all[:, g, oc * OC:(oc + 1) * OC],
                            start=(gi == 0), stop=(gi == len(gsA) - 1))
                pa = parts.tile([128, D], f32, tag="pa", name="pa")
                for oc in range(NOC):
                    nc.scalar.activation(pa[:, oc * OC:(oc + 1) * OC], pp[oc],
                                         ACT.Copy)
                partA.append(pa)
            for rt in range(NRT):
                pp = [psP.tile([128, OC], f32, tag=f"ppo{oc}", name=f"ppo{oc}")
                      for oc in range(NOC)]
                for gi, g in enumerate(gsB):
                    al = attn_all[:, g, rt * 128:(rt + 1) * 128]
                    for oc in range(NOC):
                        nc.tensor.matmul(
                            pp[oc], lhsT=al,
                            rhs=wo_all[:, g, oc * OC:(oc + 1) * OC],
                            start=(gi == 0), stop=(gi == len(gsB) - 1))
                osb = outp.tile([128, D], bf16, tag="osb", name="osb")
                for oc in range(NOC):
                    nc.vector.tensor_add(osb[:, oc * OC:(oc + 1) * OC],
                                         pp[oc],
                                         partA[rt][:, oc * OC:(oc + 1) * OC])
                    nc.sync.dma_start(
                        out=out[rt * 128:(rt + 1) * 128, oc * OC:(oc + 1) * OC],
                        in_=osb[:, oc * OC:(oc + 1) * OC])

    nc.compile()
    return nc


def make_in_maps(x, cos, sin, Wq, Wk, Wv, Wo, QC):
    import ml_dtypes
    bf = ml_dtypes.bfloat16
    B, S, D = x.shape
    HD = cos.shape[1]
    H = Wq.shape[1] // HD
    QH = H // CORES
    NT = QC // 128
    R = B * S
    DK = D // 128
    RC = 512
    NCH = R // RC

    # x packed so each 512-row chunk is contiguous per partition:
    # xP[p, n, kt, c] = x[n*RC+c, kt*128+p]
    xf = np.asarray(x, dtype=np.float32).reshape(R, D)
    xPf = np.ascontiguousarray(
        xf.reshape(NCH, RC, DK, 128).transpose(3, 0, 2, 1)).astype(bf)

    cosT = np.ascontiguousarray(cos.T).astype(bf)
    sT = sin.T.astype(np.float32)
    half = HD // 2
    sinTs = np.ascontiguousarray(
        np.concatenate([-sT[:half], sT[half:]], axis=0)).astype(bf)

    # 0/1 causal masks (applied post-exp as a multiplicative mask on DVE)
    mk = np.zeros((128, NT * QC), dtype=np.float32)
    kk = np.arange(128)[:, None]
    qq = np.arange(QC)[None, :]
    for j in range(NT):
        mk[:, j * QC:(j + 1) * QC] = np.where(qq >= j * 128 + kk, 1.0, 0.0)
    mk = mk.astype(bf)
    ident = np.eye(128, dtype=np.float32).astype(bf)

    def pack_w(w):  # [D, C] -> [128, DK, C]
        C = w.shape[1]
        return np.ascontiguousarray(
            np.asarray(w, np.float32).reshape(DK, 128, C).transpose(1, 0, 2))

    in_maps = []
    for c in range(CORES):
        wq_c = Wq[:, c * QH * HD:(c + 1) * QH * HD]
        wk_c = Wk[:, c * HD:(c + 1) * HD]
        wv_c = Wv[:, c * HD:(c + 1) * HD]
        wcat = np.concatenate(
            [np.asarray(wq_c, np.float32), np.asarray(wk_c, np.float32),
             np.asarray(wv_c, np.float32)], axis=1)
        in_maps.append({
            "xP": xPf,
            "cosT": cosT,
            "sinT": sinTs,
            "wPack": pack_w(wcat).astype(bf),
            "wo": np.asarray(Wo).astype(bf),
            "masks": mk,
            "ident": ident,
            "ones": np.ones((128, 128), dtype=bf),
        })
    return in_maps


def _install_profile_shim():
    """Provide antenv.axon_hooks (missing in this image) so
    run_bass_kernel_spmd(trace=True) can capture NTFF profiles via the
    axon PJRT .so; also neuter the artifact upload."""
    import types

    try:
        import antenv.axon_hooks  # noqa: F401
    except ImportError:
        from trn_agent_boot.trn_boot import _ntff_profile_via_ctypes
        hook = _ntff_profile_via_ctypes("/opt/axon/libaxon_pjrt.so")
        if hook is None:
            raise RuntimeError("libaxon_pjrt.so lacks profile symbols")
        mod = types.ModuleType("antenv.axon_hooks")
        mod.get_axon_ntff_profile_hook = lambda: hook
        mod.set_axon_ntff_profile_hook = lambda h: None
        sys.modules["antenv.axon_hooks"] = mod
        import antenv
        antenv.axon_hooks = mod
    import concourse.bass_utils as bu
    bu.upload_artifacts = lambda tmpdir: str(tmpdir)


_NC_CACHE = {}


def _get_nc(B, S, D, H, KV, HD, HO, QC):
    key = (B, S, D, H, KV, HD, HO, QC)
    if key not in _NC_CACHE:
        _NC_CACHE[key] = build_nc(B, S, D, H, KV, HD, HO, QC)
    return _NC_CACHE[key]


def kernel(x, cos, sin, Wq, Wk, Wv, Wo, _sim=False):
    x = np.asarray(x, dtype=np.float32)
    cos = np.asarray(cos, dtype=np.float32)
    sin = np.asarray(sin, dtype=np.float32)
    Wq = np.asarray(Wq, dtype=np.float32)
    Wk = np.asarray(Wk, dtype=np.float32)
    Wv = np.asarray(Wv, dtype=np.float32)
    Wo = np.asarray(Wo, dtype=np.float32)

    B, S, D = x.shape
    HD = cos.shape[1]
    H = Wq.shape[1] // HD
    KV = Wk.shape[1] // HD
    HO = Wq.shape[1]
    R = B * S
    QC = R // CORES

    nc = _get_nc(B, S, D, H, KV, HD, HO, QC)
    in_maps = make_in_maps(x, cos, sin, Wq, Wk, Wv, Wo, QC)

    if _sim:
        from concourse import bass_interp
        sim = bass_interp.MultiCoreSim(nc, CORES)
        for c in range(CORES):
            for k, v in in_maps[c].items():
                sim.cores[c].tensor(k)[:] = v
        sim.simulate(check_with_hw=False)
        shards = [np.array(sim.cores[c].mem_tensor("out")) for c in range(CORES)]
    else:
        from concourse.bass_utils import run_bass_kernel_spmd
        trace = os.environ.get("KERNEL_TRACE", "0") == "1"
        res = None
        if trace:
            try:
                _install_profile_shim()
                tmpdir = os.environ.get("KERNEL_TMPDIR") or None
                res = run_bass_kernel_spmd(nc, in_maps,
                                           core_ids=list(range(CORES)),
                                           trace=True, tmpdir=tmpdir)
            except Exception as e:  # fall back to untraced run
                print(f"traced run failed ({type(e).__name__}: {e}); "
                      f"retrying untraced")
                res = None
        if res is None:
            res = run_bass_kernel_spmd(nc, in_maps,
                                       core_ids=list(range(CORES)),
                                       trace=False)
        if res.exec_time_ns is not None:
            print(f"HW exec time: {res.exec_time_ns} ns")
        shards = [res.results[c]["out"] for c in range(CORES)]

    return np.concatenate(
        [np.asarray(s, dtype=np.float32) for s in shards],
        axis=0).reshape(B, S, D)


# revision 43
# speedup vs baseline: 1.0337x; 1.0337x over previous
"""GQA causal attention (RoPE, B=2 S=2048 D=2048 H=16 KV=8 HD=128) on 8 trn2 cores.

Strategy: head-parallel. Each core c owns q-heads {2c, 2c+1} and kv-head c.
Host replicates x (packed per-chunk contiguous, bf16) to all cores; all
projections, RoPE and causal attention are head-sharded (zero comm). Two
AllToAlls (one per local q-head, 1 MB/rank each, bf16) convert the attention
output from head-sharding to row-sharding overlapped with the other head's
attention, then each core computes its 512-row slice of the output projection
with the full Wo. Host concatenates the 8 row shards.

Perf notes (450us baseline -> ~361us):
- All inputs are host-packed so each DMA has multi-KB contiguous lines
  (descriptor-rate, ~43ns/1KB-line/engine, was the phase-1 bottleneck).
- Attention is software-pipelined: the PV matmuls of iteration i-1 are woven
  tile-by-tile between the score matmuls of iteration i so the scalar
  engine's exp keeps up and PSUM score banks recycle in time. qc runs
  ascending on even b / descending on odd b so adjacent iterations have
  matched tile counts (no bare score runs at block boundaries).
- The softmax denominator is a bf16 tree-sum of prob tiles on the (otherwise
  idle) vector engine plus one short ones-matmul per chunk; the causal mask
  is applied post-exp as a 0/1 multiply on DVE (raw masked-region scores are
  bounded, exp cannot overflow) - both remove ~90k cycles of PE work.
- Output projection accumulates the 8 heads from AllToAll#0 for all four row
  tiles first (hiding AllToAll#1 behind ~27us of PE work), drains partials
  to SBUF, then adds the second-half chains with a DVE combine.
- The PE is power-throttled (GPIO brake) to ~1.2-2GHz under sustained load,
  and cross-core launch skew (20-50us) surfaces as AllToAll wait time;
  both cap further gains from scheduling alone.
"""

import os
import sys

import numpy as np

if "/opt/trn_rl_repo" not in sys.path:
    sys.path.insert(0, "/opt/trn_rl_repo")

CORES = 8


def build_nc(B, S, D, H, KV, HD, HO, QC):
    """Build the SPMD bass graph (same graph for all 8 cores)."""
    import concourse.bacc as bacc
    import concourse.tile as tile
    from concourse import mybir
    from contextlib import ExitStack

    f32 = mybir.dt.float32
    bf16 = mybir.dt.bfloat16
    ACT = mybir.ActivationFunctionType

    QH = H // CORES               # q heads per core (2)
    R = B * S                     # total rows (4096)
    RO = R // CORES               # output rows per core (512) == QC
    assert QC == RO
    DK = D // 128                 # k-tiles over model dim (16)
    RC = 512                      # row-chunk width for projections
    NCH = R // RC                 # projection row chunks (8)
    NQC = S // QC                 # q chunks per batch (4)
    NKT = S // 128                # k tiles per batch (16)
    NT = QC // 128                # diagonal mask patterns (4)
    NRT = RO // 128               # out row tiles per core (4)
    OC = min(D, 512)              # out col chunk
    NOC = D // OC                 # out col chunks (4)
    HG = H                        # total heads in O-proj
    WC = QH * HD + 2 * HD         # packed projection cols (512)
    scale = float(HD) ** -0.5

    nc = bacc.Bacc("TRN2", target_bir_lowering=False, debug=False,
                   num_devices=CORES)

    xP = nc.dram_tensor("xP", [128, NCH, DK, RC], bf16, kind="ExternalInput")
    cosT = nc.dram_tensor("cosT", [HD, S], bf16, kind="ExternalInput")
    sinT = nc.dram_tensor("sinT", [HD, S], bf16, kind="ExternalInput")
    wPack = nc.dram_tensor("wPack", [128, DK, WC], bf16, kind="ExternalInput")
    wo = nc.dram_tensor("wo", [HO, D], bf16, kind="ExternalInput")
    masks = nc.dram_tensor("masks", [128, NT * QC], bf16, kind="ExternalInput")
    ident = nc.dram_tensor("ident", [128, 128], bf16, kind="ExternalInput")
    ones = nc.dram_tensor("ones", [128, 128], bf16, kind="ExternalInput")
    out = nc.dram_tensor("out", [RO, D], bf16, kind="ExternalOutput")

    with tile.TileContext(nc) as tc, ExitStack() as top:
        dram = top.enter_context(tc.tile_pool(name="dram", bufs=1, space="DRAM"))
        consts = top.enter_context(tc.tile_pool(name="consts", bufs=1))
        resid = top.enter_context(tc.tile_pool(name="resid", bufs=1))

        a2a_in = [dram.tile([CORES, 128, QC], bf16, name=f"a2ain{h}")
                  for h in range(QH)]
        a2a_out = [dram.tile([CORES, 128, QC], bf16, name=f"a2aout{h}")
                   for h in range(QH)]

        ident_sb = consts.tile([128, 128], bf16)
        ones_sb = consts.tile([128, 128], bf16)
        mask_sb = consts.tile([128, NT * QC], bf16)

        # residents produced by projection phase, consumed by attention
        qT_sb = resid.tile([128, QH, R], bf16)       # [hd, head, row]
        kT_sb = resid.tile([128, R], bf16)           # [hd, row]
        v_sb = resid.tile([128, R], bf16)            # [kpos%128, ktile*HD+hd]

        # full Wo resident (DMA'd during phase 2; no contention with x)
        wo_all = resid.tile([128, HG, D], bf16)
        attn_all = resid.tile([128, HG, QC], bf16)

        # ------------------------------- phase 1: projections + rope
        with ExitStack() as ph1:
            ropec = ph1.enter_context(tc.tile_pool(name="ropec", bufs=1))
            wpool = ph1.enter_context(tc.tile_pool(name="wpool", bufs=1))
            xpool = ph1.enter_context(tc.tile_pool(name="xpool", bufs=3))
            rtmp = ph1.enter_context(tc.tile_pool(name="rtmp", bufs=2))
            vtp = ph1.enter_context(tc.tile_pool(name="vtp", bufs=2))
            psA = ph1.enter_context(tc.tile_pool(name="psA", bufs=4, space="PSUM"))
            psTR = ph1.enter_context(tc.tile_pool(name="psTR", bufs=2, space="PSUM"))

            w_sb = wpool.tile([128, DK, WC], bf16)
            cos_sb = ropec.tile([128, S], bf16)
            sin_sb = ropec.tile([128, S], bf16)

            # chunk 0 + weights interleaved in quarter-pieces so the first
            # matmul can start after ~2 pieces land (subtile deps); few
            # trigger instructions keep the sync queue ahead of the PE
            G4 = max(DK // 4, 1)
            xch0 = xpool.tile([128, DK, RC], bf16, tag="xch", name="xch0")
            for gi, g in enumerate(range(0, DK, G4)):
                sl = slice(g, g + G4)
                nc.sync.dma_start(out=w_sb[:, sl, :], in_=wPack[:, sl, :])
                nc.sync.dma_start(out=xch0[:, sl, :], in_=xP[:, 0, sl, :])
                if gi == 1:
                    nc.sync.dma_start(out=cos_sb, in_=cosT[:, :])
                    nc.sync.dma_start(out=sin_sb, in_=sinT[:, :])
            nc.sync.dma_start(out=ident_sb, in_=ident[:, :])
            # remaining chunks: one DMA each (16KB contiguous lines);
            # Wo trickles in behind them (keeps the a2a window DMA-quiet)
            wo_r = wo.ap().rearrange("(g p) n -> p g n", p=128)
            wo_done = set()
            xchs = [xch0]
            for n in range(1, NCH):
                xc = xpool.tile([128, DK, RC], bf16, tag="xch", name="xch")
                nc.sync.dma_start(out=xc, in_=xP[:, n, :, :])
                xchs.append(xc)
                if n == min(2, NCH - 1):
                    nc.sync.dma_start(out=ones_sb, in_=ones[:, :])
                    nc.sync.dma_start(out=mask_sb, in_=masks[:, :])
                # last two Wo quarters ride behind the final x chunks (x DMA
                # traffic has drained by then); the rest go at phase-2 start
                q4 = n - (NCH - 2)
                if 0 <= q4 < 2:
                    wo_done.add(q4)
                    sl = slice(q4 * HG // 4, (q4 + 1) * HG // 4)
                    nc.sync.dma_start(out=wo_all[:, sl, :], in_=wo_r[:, sl, :])
            wo_rest = [q4 for q4 in range(4) if q4 not in wo_done]

            half = HD // 2

            def rope(pp, dst, poff):
                c_sl = cos_sb[:, poff:poff + RC]
                s_sl = sin_sb[:, poff:poff + RC]
                t1 = rtmp.tile([128, RC], f32, tag="t1", name="t1")
                t2 = rtmp.tile([128, RC], f32, tag="t2", name="t2")
                nc.vector.tensor_mul(t1, pp, c_sl)
                nc.vector.tensor_mul(t2[0:half, :], pp[half:128, :], s_sl[0:half, :])
                nc.vector.tensor_mul(t2[half:128, :], pp[0:half, :], s_sl[half:128, :])
                nc.vector.tensor_add(dst, t1, t2)

            pend_tr = []   # deferred PE transposes from the previous chunk
            for n in range(NCH):
                xs = xchs[n]
                poff = (n * RC) % S
                ois = list(range(QH + 2))  # QH q heads, then k, then vT
                if n == NCH - 1:
                    # last chunk: v first so its transposes (emitted at loop
                    # end) don't stall the PE at the phase boundary
                    ois = [QH + 1] + ois[:-1]
                for ji, oi in enumerate(ois):
                    pp = psA.tile([128, RC], f32, tag="pp", name="pp")
                    coff = oi * HD if oi < QH else (QH + (oi - QH)) * HD
                    for kt in range(DK):
                        nc.tensor.matmul(
                            pp, lhsT=w_sb[:, kt, coff:coff + HD],
                            rhs=xs[:, kt, :],
                            start=(kt == 0), stop=(kt == DK - 1))
                    if ji == 0 and pend_tr:
                        for t in pend_tr:
                            t()
                        pend_tr = []
                    if oi < QH:
                        rope(pp, qT_sb[:, oi, n * RC:(n + 1) * RC], poff)
                    elif oi == QH:
                        rope(pp, kT_sb[:, n * RC:(n + 1) * RC], poff)
                    else:
                        vt_sb = vtp.tile([128, RC], bf16, tag="vt", name="vt")
                        nc.scalar.activation(vt_sb, pp, ACT.Copy)

                        def mk(nn, vt):
                            def emit():
                                for j in range(RC // 128):
                                    ptr_ = psTR.tile([128, 128], bf16,
                                                     tag="ptr", name="ptr")
                                    nc.tensor.transpose(
                                        ptr_, vt[:, j * 128:(j + 1) * 128],
                                        ident_sb)
                                    rti = nn * (RC // 128) + j
                                    nc.scalar.activation(
                                        v_sb[:, rti * 128:(rti + 1) * 128],
                                        ptr_, ACT.Copy)
                            return emit
                        pend_tr.append(mk(n, vt_sb))
            for t in pend_tr:
                t()
            pend_tr = []

        # ------------------------------- phase 2: attention, pipelined
        with ExitStack() as ph2:
            probs = ph2.enter_context(tc.tile_pool(name="probs", bufs=46))
            atmp = ph2.enter_context(tc.tile_pool(name="atmp", bufs=3))
            dens = ph2.enter_context(tc.tile_pool(name="dens", bufs=2))
            accp = ph2.enter_context(tc.tile_pool(name="accp", bufs=4))
            psS = ph2.enter_context(tc.tile_pool(name="psS", bufs=3, space="PSUM"))
            psO = ph2.enter_context(tc.tile_pool(name="psO", bufs=3, space="PSUM"))
            psD = ph2.enter_context(tc.tile_pool(name="psD", bufs=1, space="PSUM"))
            psB = ph2.enter_context(tc.tile_pool(name="psB", bufs=1, space="PSUM"))

            from concourse import mybir as _mb

            for q4 in wo_rest:   # remaining Wo quarters (x traffic is done)
                sl = slice(q4 * HG // 4, (q4 + 1) * HG // 4)
                nc.sync.dma_start(out=wo_all[:, sl, :], in_=wo_r[:, sl, :])

            # qc ascending on even b, descending on odd b: adjacent
            # iterations then have matched tile counts (the PV weave of the
            # previous iteration fills every score position, so exp on the
            # scalar engine never falls behind), while each h still ends on
            # a small qc for a fast AllToAll trigger.
            iters = []
            for h in range(QH):
                for b in range(B):
                    qcs = (range(NQC) if b % 2 == 0
                           else range(NQC - 1, -1, -1))
                    iters.extend((h, b, qc) for qc in qcs)

            def score_tile(it, kt):
                """Emit the score matmul, exp, post-exp mask (DVE), and the
                DVE acc-chain add for tile kt of it.

                Unmasked scores are bounded (|s*scale| < ~6 for these
                magnitudes) so exp can run on the raw scores; the causal mask
                is applied post-exp as a 0/1 multiply on the vector engine,
                keeping the PE free of mask matmuls."""
                h, b, qc = it["h"], it["b"], it["qc"]
                dj = kt - qc * NT
                o = max(dj, 0) * 128
                kl = kT_sb[:, b * S + kt * 128: b * S + (kt + 1) * 128]
                sc = psS.tile([128, QC], f32, tag="sc", name="sc")
                nc.tensor.matmul(
                    sc[:, o:QC], lhsT=kl,
                    rhs=qT_sb[:, h, b * S + qc * QC + o: b * S + (qc + 1) * QC],
                    start=True, stop=True)
                pr = probs.tile([128, QC], bf16, tag="pr", name="pr")
                nc.scalar.activation(pr[:, o:QC], sc[:, o:QC], ACT.Exp,
                                     scale=scale)
                if dj >= 0:
                    prm = probs.tile([128, QC], bf16, tag="pr", name="prm")
                    nc.vector.tensor_mul(prm[:, o:QC], pr[:, o:QC],
                                         mask_sb[:, dj * QC + o:(dj + 1) * QC])
                    pr = prm
                it["prs"][kt] = pr
                it["offs"][kt] = o
                # denominator partial sums on DVE (bf16, 2x rate):
                # acc[kp, q] = sum_kt pr_kt[kp, q]; final row-sum is one
                # short PE matmul per chunk.
                if kt == 0:
                    it["acc"] = pr          # region [0:QC] valid (o == 0)
                else:
                    po_ = it["offs"][kt]
                    na = accp.tile([128, QC], bf16, tag="acc", name="acc")
                    if po_ > 0:
                        nc.vector.tensor_copy(na[:, 0:po_], it["acc"][:, 0:po_])
                    nc.vector.tensor_add(na[:, po_:QC], it["acc"][:, po_:QC],
                                         pr[:, po_:QC])
                    it["acc"] = na

            def pv_tile(it, kt):
                """Emit the PV matmul for tile kt of it."""
                b, nkt = it["b"], it["nkt"]
                o = it["offs"][kt]
                ktg = b * NKT + kt
                nc.tensor.matmul(
                    it["po"][:, o:QC], lhsT=v_sb[:, ktg * 128:(ktg + 1) * 128],
                    rhs=it["prs"][kt][:, o:QC],
                    start=(kt == 0), stop=(kt == nkt - 1))

            def den_stage(it):
                """One short den matmul on the acc, then a bf16 round of the
                denominator. Everything off the scalar engine: ACT must only
                run exp, or the copies delay the exp stream and stall the PV
                weave. The reciprocal happens AFTER the broadcast (on the
                [128, QC] tile) so the whole chain is one DVE op shorter."""
                pden = psD.tile([1, QC], f32, tag="pden", name="pden")
                nc.tensor.matmul(pden, lhsT=ones_sb[:, 0:1], rhs=it["acc"],
                                 start=True, stop=True)
                den_b = dens.tile([1, QC], bf16, tag="denb", name="den_b")
                nc.vector.tensor_copy(den_b, pden)
                it["den_b"] = den_b

            def norm_stage(it):
                """broadcast den (PE) -> reciprocal on the broadcast (DVE)."""
                pbc = psB.tile([128, QC], f32, tag="pbc", name="pbc")
                nc.tensor.matmul(pbc, lhsT=ones_sb[0:1, :], rhs=it["den_b"],
                                 start=True, stop=True)
                rec = atmp.tile([128, QC], f32, tag="rec", name="rec")
                nc.vector.reciprocal_approx_fast(rec, pbc)
                it["rec"] = rec

            def norm_stage2(it):
                """scale po by 1/den (DVE) -> a2a-in DMA."""
                h, b, qc = it["h"], it["b"], it["qc"]
                anorm = atmp.tile([128, QC], bf16, tag="an", name="anorm")
                nc.vector.tensor_mul(anorm, it["po"], it["rec"])
                d = b * NQC + qc
                nc.sync.dma_start(out=a2a_in[h][d], in_=anorm)
                if it["h_last"]:
                    finish_h(h)

            def finish_h(h):
                nc.gpsimd.collective_compute(
                    "AllToAll", _mb.AluOpType.bypass,
                    ins=[a2a_in[h].opt()], outs=[a2a_out[h].opt()],
                    replica_groups=[list(range(CORES))])
                asrc = a2a_out[h].rearrange("g p q -> p g q")
                adst = attn_all.rearrange("p (g hl) q -> p g hl q", hl=QH)
                for q8 in range(min(8, CORES)):
                    gs = max(CORES // 8, 1)
                    sl = slice(q8 * gs, (q8 + 1) * gs)
                    if sl.start >= CORES:
                        break
                    nc.sync.dma_start(out=adst[:, sl, h, :], in_=asrc[:, sl, :])

            prev = None   # iteration whose PV/den/norm weave into the current
            for ii, (h, b, qc) in enumerate(iters):
                cur = {"h": h, "b": b, "qc": qc, "nkt": (qc + 1) * NT,
                       "prs": {}, "offs": {},
                       "h_last": (ii == len(iters) - 1
                                  or iters[ii + 1][0] != h),
                       "po": psO.tile([128, QC], f32, tag="po", name="po")}
                nw = max(cur["nkt"], prev["nkt"] if prev else 0)
                den_done = norm1_done = norm2_done = prev is None
                for i in range(nw):
                    if i < cur["nkt"]:
                        score_tile(cur, i)
                    if prev and i < prev["nkt"]:
                        pv_tile(prev, i)
                    # i == 4: prev's final acc-add needs ~1-2us to clear the
                    # DVE queue at weave start; an earlier den matmul waits
                    if i == 4 and not den_done:
                        den_stage(prev)
                        den_done = True
                    if den_done and not norm1_done and i >= 6:
                        norm_stage(prev)
                        norm1_done = True
                    # the final scale may only run once prev's PV chain has
                    # fully accumulated (pv_tile(prev, nkt-1) emitted above)
                    if (norm1_done and not norm2_done and prev
                            and i >= max(prev["nkt"] - 1, 8)):
                        norm_stage2(prev)
                        norm2_done = True
                if not den_done:
                    den_stage(prev)
                if not norm1_done:
                    norm_stage(prev)
                if not norm2_done:
                    norm_stage2(prev)
                prev = cur
            # flush: PV of the last iteration, then its den/norm
            for i in range(prev["nkt"]):
                pv_tile(prev, i)
            den_stage(prev)
            norm_stage(prev)
            norm_stage2(prev)

        # ------------------------------- phase 3: output projection
        # Pass A: accumulate the QH*? heads delivered by AllToAll#0 for all
        # row tiles (this work hides AllToAll#1), drain partials to SBUF.
        # Pass B: second-half chains + DVE combine -> bf16 out.
        with ExitStack() as ph3:
            parts = ph3.enter_context(tc.tile_pool(name="parts", bufs=NRT))
            outp = ph3.enter_context(tc.tile_pool(name="outp", bufs=2))
            psP = ph3.enter_context(tc.tile_pool(name="psP", bufs=2, space="PSUM"))

            gsA = [g for g in range(HG) if g % QH == 0]
            gsB = [g for g in range(HG) if g % QH != 0]
            partA = []
            for rt in range(NRT):
                pp = [psP.tile([128, OC], f32, tag=f"ppo{oc}", name=f"ppo{oc}")
                      for oc in range(NOC)]
                for gi, g in enumerate(gsA):
                    al = attn_all[:, g, rt * 128:(rt + 1) * 128]
                    for oc in range(NOC):
                        nc.tensor.matmul(
                            pp[oc], lhsT=al,
                            rhs=wo_all[:, g, oc * OC:(oc + 1) * OC],
                            start=(gi == 0), stop=(gi == len(gsA) - 1))
                pa = parts.tile([128, D], f32, tag="pa", name="pa")
                for oc in range(NOC):
                    nc.scalar.activation(pa[:, oc * OC:(oc + 1) * OC], pp[oc],
                                         ACT.Copy)
                partA.append(pa)
            for rt in range(NRT):
                pp = [psP.tile([128, OC], f32, tag=f"ppo{oc}", name=f"ppo{oc}")
                      for oc in range(NOC)]
                for gi, g in enumerate(gsB):
                    al = attn_all[:, g, rt * 128:(rt + 1) * 128]
                    for oc in range(NOC):
                        nc.tensor.matmul(
                            pp[oc], lhsT=al,
                            rhs=wo_all[:, g, oc * OC:(oc + 1) * OC],
                            start=(gi == 0), stop=(gi == len(gsB) - 1))
                osb = outp.tile([128, D], bf16, tag="osb", name="osb")
                for oc in range(NOC):
                    nc.vector.tensor_add(osb[:, oc * OC:(oc + 1) * OC],
                                         pp[oc],
                                         partA[rt][:, oc * OC:(oc + 1) * OC])
                    nc.sync.dma_start(
                        out=out[rt * 128:(rt + 1) * 128, oc * OC:(oc + 1) * OC],
                        in_=osb[:, oc * OC:(oc + 1) * OC])

    nc.compile()
    return nc


def make_in_maps(x, cos, sin, Wq, Wk, Wv, Wo, QC):
    import ml_dtypes
    bf = ml_dtypes.bfloat16
    B, S, D = x.shape
    HD = cos.shape[1]
    H = Wq.shape[1] // HD
    QH = H // CORES
    NT = QC // 128
    R = B * S
    DK = D // 128
    RC = 512
    NCH = R // RC

    # x packed so each 512-row chunk is contiguous per partition:
    # xP[p, n, kt, c] = x[n*RC+c, kt*128+p]
    xf = np.asarray(x, dtype=np.float32).reshape(R, D)
    xPf = np.ascontiguousarray(
        xf.reshape(NCH, RC, DK, 128).transpose(3, 0, 2, 1)).astype(bf)

    cosT = np.ascontiguousarray(cos.T).astype(bf)
    sT = sin.T.astype(np.float32)
    half = HD // 2
    sinTs = np.ascontiguousarray(
        np.concatenate([-sT[:half], sT[half:]], axis=0)).astype(bf)

    # 0/1 causal masks (applied post-exp as a multiplicative mask on DVE)
    mk = np.zeros((128, NT * QC), dtype=np.float32)
    kk = np.arange(128)[:, None]
    qq = np.arange(QC)[None, :]
    for j in range(NT):
        mk[:, j * QC:(j + 1) * QC] = np.where(qq >= j * 128 + kk, 1.0, 0.0)
    mk = mk.astype(bf)
    ident = np.eye(128, dtype=np.float32).astype(bf)

    def pack_w(w):  # [D, C] -> [128, DK, C]
        C = w.shape[1]
        return np.ascontiguousarray(
            np.asarray(w, np.float32).reshape(DK, 128, C).transpose(1, 0, 2))

    in_maps = []
    for c in range(CORES):
        wq_c = Wq[:, c * QH * HD:(c + 1) * QH * HD]
        wk_c = Wk[:, c * HD:(c + 1) * HD]
        wv_c = Wv[:, c * HD:(c + 1) * HD]
        wcat = np.concatenate(
            [np.asarray(wq_c, np.float32), np.asarray(wk_c, np.float32),
             np.asarray(wv_c, np.float32)], axis=1)
        in_maps.append({
            "xP": xPf,
            "cosT": cosT,
            "sinT": sinTs,
            "wPack": pack_w(wcat).astype(bf),
            "wo": np.asarray(Wo).astype(bf),
            "masks": mk,
            "ident": ident,
            "ones": np.ones((128, 128), dtype=bf),
        })
    return in_maps


def _install_profile_shim():
    """Provide antenv.axon_hooks (missing in this image) so
    run_bass_kernel_spmd(trace=True) can capture NTFF profiles via the
    axon PJRT .so; also neuter the artifact upload."""
    import types

    try:
        import antenv.axon_hooks  # noqa: F401
    except ImportError:
        from trn_agent_boot.trn_boot import _ntff_profile_via_ctypes
        hook = _ntff_profile_via_ctypes("/opt/axon/libaxon_pjrt.so")
        if hook is None:
            raise RuntimeError("libaxon_pjrt.so lacks profile symbols")
        mod = types.ModuleType("antenv.axon_hooks")
        mod.get_axon_ntff_profile_hook = lambda: hook
        mod.set_axon_ntff_profile_hook = lambda h: None
        sys.modules["antenv.axon_hooks"] = mod
        import antenv
        antenv.axon_hooks = mod
    import concourse.bass_utils as bu
    bu.upload_artifacts = lambda tmpdir: str(tmpdir)


_NC_CACHE = {}


def _get_nc(B, S, D, H, KV, HD, HO, QC):
    key = (B, S, D, H, KV, HD, HO, QC)
    if key not in _NC_CACHE:
        _NC_CACHE[key] = build_nc(B, S, D, H, KV, HD, HO, QC)
    return _NC_CACHE[key]


def kernel(x, cos, sin, Wq, Wk, Wv, Wo, _sim=False):
    x = np.asarray(x, dtype=np.float32)
    cos = np.asarray(cos, dtype=np.float32)
    sin = np.asarray(sin, dtype=np.float32)
    Wq = np.asarray(Wq, dtype=np.float32)
    Wk = np.asarray(Wk, dtype=np.float32)
    Wv = np.asarray(Wv, dtype=np.float32)
    Wo = np.asarray(Wo, dtype=np.float32)

    B, S, D = x.shape
    HD = cos.shape[1]
    H = Wq.shape[1] // HD
    KV = Wk.shape[1] // HD
    HO = Wq.shape[1]
    R = B * S
    QC = R // CORES

    nc = _get_nc(B, S, D, H, KV, HD, HO, QC)
    in_maps = make_in_maps(x, cos, sin, Wq, Wk, Wv, Wo, QC)

    if _sim:
        from concourse import bass_interp
        sim = bass_interp.MultiCoreSim(nc, CORES)
        for c in range(CORES):
            for k, v in in_maps[c].items():
                sim.cores[c].tensor(k)[:] = v
        sim.simulate(check_with_hw=False)
        shards = [np.array(sim.cores[c].mem_tensor("out")) for c in range(CORES)]
    else:
        from concourse.bass_utils import run_bass_kernel_spmd
        trace = os.environ.get("KERNEL_TRACE", "0") == "1"
        res = None
        if trace:
            try:
                _install_profile_shim()
                tmpdir = os.environ.get("KERNEL_TMPDIR") or None
                res = run_bass_kernel_spmd(nc, in_maps,
                                           core_ids=list(range(CORES)),
                                           trace=True, tmpdir=tmpdir)
            except Exception as e:  # fall back to untraced run
                print(f"traced run failed ({type(e).__name__}: {e}); "
                      f"retrying untraced")
                res = None
        if res is None:
            res = run_bass_kernel_spmd(nc, in_maps,
                                       core_ids=list(range(CORES)),
                                       trace=False)
        if res.exec_time_ns is not None:
            print(f"HW exec time: {res.exec_time_ns} ns")
        shards = [res.results[c]["out"] for c in range(CORES)]

    return np.concatenate(
        [np.asarray(s, dtype=np.float32) for s in shards],
        axis=0).reshape(B, S, D)
